# revision 24
# baseline (speedup 1.0000x reference)
"""Trainium2 Bass kernel for a pre-LN transformer block (causal MHA + GELU FFN).

Problem: x[64, 512, 384], 7 heads x 54, FFN 2304. Sharded data-parallel over
batch across 8 NeuronCores (8 batches/core); no collectives needed.

v1 (bf16 rewrite of the fp32r baseline):
  - all matmul operands bf16 (fp32 accumulate in PSUM); rel-err budget 2e-2
  - LN rsqrt via exp(-0.5*ln(var+eps)) so the ACT engine table set stays on
    {ln,exp} for the whole attention phase and {gelu} for the FFN phase
    (2 table loads per rep instead of ~3 per batch)
  - per-head scores live in a 2-bank bf16 PSUM tile packed causally:
    4 diagonal 128-blocks at cols 0..512, rectangles at 512..896 /
    1024..1280 / 1280..1408; one exp() call [0..1408]; one 4-block
    triangular mask multiply on the diagonal region
  - x2 (post-attention residual) persists in SBUF; x is loaded once
  - FFN2 accumulates into the (phase-A scores) PSUM tiles bitcast to f32
"""

import numpy as np
from contextlib import ExitStack

import concourse.bass as bass
import concourse.bacc as bacc
import concourse.mybir as mybir
import concourse.tile as tile
from concourse import masks
from concourse.bass_utils import run_bass_kernel_spmd

# ---- problem constants (hardcoded per harness contract) ----
B, S, D = 64, 512, 384
H, HS = 7, 54
FFN = 6 * D  # 2304
EPS = 1e-5
N_CORES = 8
B_LOC = B // N_CORES          # 8 batches per core
F32 = mybir.dt.float32
BF16 = mybir.dt.bfloat16
AF = mybir.ActivationFunctionType
ALU = mybir.AluOpType

N_D = D // 128                # 3 d-chunks
N_F = FFN // 128              # 18 ffn-chunks
PAIRS = (H + 1) // 2          # 4 head-pair groups (last has 1 head)

# packed causal-score layout inside a [128, 1536] f32 PSUM tile (3 banks;
# f32 bank = 512 elements -> matmul outputs must not cross 512-col lines).
# chunk j holds scores for key-block j vs queries s in [128j, 512) --
# width 512-128j, diagonal 128-block at the chunk start:
#   j0 [0..512) bank0 | j1 [512..896) + j3 [896..1024) bank1 | j2 [1024..1280) bank2
P_OFF = (0, 512, 1024, 896)
P_W = (512, 384, 256, 128)
EXP_END = 1280                # exp() covers [0, 1280) in one call

LAST_RESULTS = None


def build_program(n_b=B_LOC, has_bias_o=False, has_bias_2=False, n_reps=1):
    nc = bacc.Bacc()
    NTOK = n_b * S

    x_d = nc.declare_dram_parameter("x", [NTOK, D], F32, isOutput=False)
    wq_d = nc.declare_dram_parameter("wq_pad", [D, 512], BF16, isOutput=False)
    wk_d = nc.declare_dram_parameter("wk_pad", [D, 512], BF16, isOutput=False)
    wv_d = nc.declare_dram_parameter("wv_pad", [D, 512], BF16, isOutput=False)
    wo_d = nc.declare_dram_parameter("wo_pad", [H, 55, D], BF16, isOutput=False)
    w1_d = nc.declare_dram_parameter("w1", [D, FFN], BF16, isOutput=False)
    w2_d = nc.declare_dram_parameter("w2", [FFN, D], BF16, isOutput=False)
    b1_d = nc.declare_dram_parameter("b1c", [128, N_F], F32, isOutput=False)
    bo_d = nc.declare_dram_parameter("bo", [1, D], BF16, isOutput=False)
    b2_d = nc.declare_dram_parameter("b2", [1, D], BF16, isOutput=False)
    out_d = nc.declare_dram_parameter("out", [NTOK, D], F32, isOutput=True)

    with tile.TileContext(nc) as tc, ExitStack() as ctx, \
            nc.allow_low_precision(reason="bf16 kernel; rel-err gate 2e-2"):
        # ---------------- persistent pools ----------------
        wpool = ctx.enter_context(tc.tile_pool(name="weights", bufs=1))
        wq_sb = [wpool.tile([128, 512], BF16, tag=f"wq{d}", name=f"wq{d}") for d in range(N_D)]
        wk_sb = [wpool.tile([128, 512], BF16, tag=f"wk{d}", name=f"wk{d}") for d in range(N_D)]
        wv_sb = [wpool.tile([128, 512], BF16, tag=f"wv{d}", name=f"wv{d}") for d in range(N_D)]
        wo_sb = [wpool.tile([55, D], BF16, tag=f"wo{h}", name=f"wo{h}") for h in range(H)]
        w1_sb = [wpool.tile([128, FFN], BF16, tag=f"w1{d}", name=f"w1{d}") for d in range(N_D)]
        w2_sb = [wpool.tile([128, D], BF16, tag=f"w2{f}", name=f"w2{f}") for f in range(N_F)]
        b1_sb = wpool.tile([128, N_F], F32, tag="b1")
        bo_sb = wpool.tile([1, D], BF16, tag="bo")
        b2_sb = wpool.tile([1, D], BF16, tag="b2")
        eps_sb = wpool.tile([128, 1], F32, tag="eps")
        ones32 = wpool.tile([1, 128], F32, tag="ones32")
        magic_sb = wpool.tile([128, 4], mybir.dt.uint32, tag="magic")
        ones_sb = wpool.tile([1, 128], BF16, tag="ones")
        tri4 = wpool.tile([128, 512], BF16, tag="tri4")
        identity = wpool.tile([128, 128], BF16, tag="ident")

        # x2 persists in SBUF between the attention and FFN phases
        x2pool = ctx.enter_context(tc.tile_pool(name="x2", bufs=1))
        x2_sb = [[x2pool.tile([128, D], F32, tag=f"x2_{b}_{j}", name=f"x2_{b}_{j}")
                  for j in range(4)] for b in range(n_b)]
        ln2_rs = [x2pool.tile([128, 4], F32, tag=f"rs2_{b}", name=f"rs2_{b}") for b in range(n_b)]
        ln2_nm = [x2pool.tile([128, 4], F32, tag=f"nm2_{b}", name=f"nm2_{b}") for b in range(n_b)]

        # ---------------- streaming pools ----------------
        xpool = ctx.enter_context(tc.tile_pool(name="xin", bufs=12))
        stpool = ctx.enter_context(tc.tile_pool(name="stats", bufs=4))
        xnpool = ctx.enter_context(tc.tile_pool(name="xn", bufs=8))
        xTpool = ctx.enter_context(tc.tile_pool(name="xT", bufs=2))
        qkpool = ctx.enter_context(tc.tile_pool(name="qk", bufs=2))
        vpool = ctx.enter_context(tc.tile_pool(name="v", bufs=8))
        epool = ctx.enter_context(tc.tile_pool(name="expT", bufs=3))
        rpool = ctx.enter_context(tc.tile_pool(name="recip", bufs=3))
        otpool = ctx.enter_context(tc.tile_pool(name="oT", bufs=2))
        hpool = ctx.enter_context(tc.tile_pool(name="hgelu", bufs=4))
        opool = ctx.enter_context(tc.tile_pool(name="outt", bufs=4))

        # prologue DMAs: batch-0 x first, then phase-A weights
        xpre = {}
        for j in range(4):
            xt = xpool.tile([128, D], F32, tag="x", name="xt")
            nc.sync.dma_start(xt[:], x_d[128 * j:128 * (j + 1), :])
            xpre[(0, j)] = xt
        for d in range(N_D):
            nc.sync.dma_start(wq_sb[d][:], wq_d[128 * d:128 * (d + 1), :])
            nc.sync.dma_start(wk_sb[d][:], wk_d[128 * d:128 * (d + 1), :])
            nc.sync.dma_start(wv_sb[d][:], wv_d[128 * d:128 * (d + 1), :])
        for h in range(H):
            nc.sync.dma_start(wo_sb[h][:], wo_d[h])
        nc.sync.dma_start(b1_sb[:], b1_d[:])
        nc.sync.dma_start(bo_sb[:], bo_d[:])
        nc.sync.dma_start(b2_sb[:], b2_d[:])
        nc.any.memset(eps_sb[:], EPS)
        nc.any.memset(magic_sb[:].bitcast(F32), np.uint32(0x5F3759DF).view(np.float32))
        nc.any.memset(ones_sb[:], 1.0)
        nc.any.memset(ones32[:], 1.0)
        masks.make_identity(nc, identity[:])
        for j in range(4):
            masks.make_upper_triangular(nc, tri4[:, 128 * j:128 * (j + 1)],
                                        val=1.0, diag=True)


        # prefetch batch-1 x ahead of the FFN weights
        if n_b > 1:
            for j in range(4):
                t0 = 128 * (4 + j)
                xt = xpool.tile([128, D], F32, tag="x", name="xt")
                nc.sync.dma_start(xt[:], x_d[t0:t0 + 128, :])
                xpre[(1, j)] = xt

        # second wave: FFN weights (queued behind phase-A essentials)
        for d in range(N_D):
            nc.sync.dma_start(w1_sb[d][:], w1_d[128 * d:128 * (d + 1), :])
        for f in range(N_F):
            nc.sync.dma_start(w2_sb[f][:], w2_d[128 * f:128 * (f + 1), :])

        dpool = ctx.enter_context(tc.tile_pool(name="dram", bufs=1, space="DRAM"))
        chain = [dpool.tile([NTOK, D], F32, tag=f"chain{i}", name=f"chain{i}")
                 for i in range(max(n_reps - 1, 0))]


        # PSUM budget (8 banks): sc 2x3 + shared proj/o 2x1
        ps_sc = ctx.enter_context(tc.tile_pool(name="ps_sc", bufs=2, space="PSUM"))
        ps_po = ctx.enter_context(tc.tile_pool(name="ps_po", bufs=2, space="PSUM"))

        def ln_stats(src_tiles, rs, nmr):
            """LN scale/shift for 4 token tiles -> rs/nmr [128, 4].
            rsqrt(var+eps) entirely on DVE (bit-trick seed + 2 Newton
            steps) so the ACT table set never changes mid-phase."""
            mv = stpool.tile([128, 8], F32, tag="mv", name="mv")
            for j in range(4):
                st6 = stpool.tile([128, 6], F32, tag="st6", name="st6")
                nc.vector.bn_stats(st6[:], src_tiles[j][:])
                nc.vector.bn_aggr(mv[:, 2 * j:2 * j + 2], st6[:])
            mv3 = mv[:].rearrange("p (j two) -> p j two", two=2)
            # rsqrt(var+eps) on DVE only: fast-inverse-sqrt seed + 2 Newton
            # steps (keeps the ACT table on {exp}/{gelu} all rep long)
            u = stpool.tile([128, 4], F32, tag="u", name="u")
            vh = stpool.tile([128, 4], F32, tag="vh", name="vh")
            yy = stpool.tile([128, 4], F32, tag="yy", name="yy")
            nc.vector.tensor_scalar(u[:], mv3[:, :, 1], EPS, None, op0=ALU.add)
            nc.vector.tensor_scalar(vh[:], u[:], 0.5, None, op0=ALU.mult)
            ui = u[:].bitcast(mybir.dt.uint32)
            nc.vector.tensor_scalar(ui, ui, 1, None, op0=ALU.logical_shift_right)
            nc.vector.tensor_tensor(rs[:].bitcast(mybir.dt.uint32), magic_sb[:],
                                    ui, op=ALU.subtract)
            for _ in range(2):
                nc.vector.tensor_mul(yy[:], rs[:], rs[:])
                nc.vector.tensor_mul(yy[:], yy[:], vh[:])
                nc.vector.tensor_scalar(yy[:], yy[:], -1.0, 1.5,
                                        op0=ALU.mult, op1=ALU.add)
                nc.vector.tensor_mul(rs[:], rs[:], yy[:])
            for j in range(4):
                # nmr = -(mu * rsig)
                nc.vector.tensor_scalar(nmr[:, j:j + 1], mv3[:, j, 0].unsqueeze(-1),
                                        rs[:, j:j + 1], -1.0,
                                        op0=ALU.mult, op1=ALU.mult)

        def apply_ln(src_tiles, rs, nmr):
            """xn = x*rs + nmr on ACT (Identity is in every table set)."""
            xn_tiles = []
            for j in range(4):
                xn = xnpool.tile([128, D], BF16, tag="xn", name="xn")
                nc.scalar.activation(xn[:], src_tiles[j][:], AF.Identity,
                                     bias=nmr[:, j:j + 1], scale=rs[:, j:j + 1])
                xn_tiles.append(xn)
            return xn_tiles

        def transpose_to_feature_major(xn_tiles, evac="dve"):
            """4x [128, D] token-major bf16 -> 3x [128, 512] feature-major."""
            xT = []
            for d in range(N_D):
                ps = ps_po.tile([128, 512], F32, tag="po", name="ps_t")
                psb = ps[:].bitcast(BF16)   # transpose out must match in dtype
                for j in range(4):
                    nc.tensor.transpose(
                        psb[:, 128 * j:128 * (j + 1)],
                        xn_tiles[j][:, 128 * d:128 * (d + 1)],
                        identity[:],
                    )
                t = xTpool.tile([128, 512], BF16, tag=f"xT{d}", name=f"xT{d}")
                if evac == "dve":
                    nc.vector.tensor_copy(t[:], psb[:, 0:512])
                else:
                    nc.scalar.copy(t[:], psb[:, 0:512])
                xT.append(t)
            return xT

        # ======================= per-rep =======================
        for rep in range(n_reps):
          xsrc_d = x_d if rep == 0 else chain[rep - 1]
          xdst_d = out_d if rep == n_reps - 1 else chain[rep]

          # ------------------- phase A: attention -------------------
          def load_x(b):
              xin = []
              for j in range(4):
                  t0 = 128 * (4 * b + j)
                  if rep == 0 and (b, j) in xpre:
                      xin.append(xpre[(b, j)])
                      continue
                  xt = xpool.tile([128, D], F32, tag="x", name="xt")
                  nc.sync.dma_start(xt[:], xsrc_d[t0:t0 + 128, :])
                  xin.append(xt)
              return xin

          def ln1_stats(xin):
              rs = stpool.tile([128, 4], F32, tag="rs", name="rs")
              nmr = stpool.tile([128, 4], F32, tag="nmr", name="nmr")
              ln_stats(xin, rs, nmr)
              return rs, nmr

          xin_b = load_x(0)
          rs_b, nmr_b = ln1_stats(xin_b)
          xn_b = apply_ln(xin_b, rs_b, nmr_b)
          for b in range(n_b):
            xin, xn_tiles = xin_b, xn_b
            xT = transpose_to_feature_major(xn_tiles)

            # Q^T / K^T per head-pair: [54, 512] at partitions 0-53/64-117
            qt, kt = [], []
            for p in range(PAIRS):
                m = 118 if p < PAIRS - 1 else 54
                for (dst_list, w_sb, tg) in ((qt, wq_sb, "q"), (kt, wk_sb, "k")):
                    ps = ps_po.tile([128, 512], F32, tag="po", name="ps_qk")
                    for d in range(N_D):
                        nc.tensor.matmul(
                            ps[0:m, :],
                            w_sb[d][:, 128 * p:128 * p + m],
                            xT[d][:],
                            start=(d == 0), stop=(d == N_D - 1),
                        )
                    t = qkpool.tile([128, 512], BF16, tag=f"{tg}{p}", name=f"{tg}{p}")
                    if tg == "q":
                        nc.scalar.copy(t[0:m, :], ps[0:m, :])
                    else:
                        nc.vector.tensor_copy(t[0:m, :], ps[0:m, :])
                    dst_list.append(t)

            # V token-major with a ones column per head
            vt = []
            for j in range(4):
                ps = ps_po.tile([128, 512], F32, tag="po", name="ps_v")
                for d in range(N_D):
                    nc.tensor.matmul(
                        ps[:],
                        xT[d][:, 128 * j:128 * (j + 1)],
                        wv_sb[d][:],
                        start=(d == 0), stop=(d == N_D - 1),
                    )
                t = vpool.tile([128, 512], BF16, tag="v", name="vt")
                nc.gpsimd.memset(t[:], 1.0)
                src = ps[:, 0:448].rearrange("p (h c) -> p h c", h=H)[:, :, 1:55]
                dst = t[:, 0:448].rearrange("p (h c) -> p h c", h=H)[:, :, 1:55]
                nc.scalar.copy(dst, src)
                vt.append(t)

            # next batch's LN1 stats: DVE is idle during this batch's heads
            if b + 1 < n_b:
                xin_b = load_x(b + 1)
                rs_b, nmr_b = ln1_stats(xin_b)

            # ---- per-head attention, software-pipelined on PE:
            #   iter h emits: scores(h) | rect-o(h-1) | diag-o(h-2)
            # so the Pool-mask latency never blocks the PE stream.
            ot_b = [None] * H
            eT_b = [None] * H
            ops_b = [None] * H
            osb_b = [None] * H
            dgrp = [rpool.tile([4, 512], F32, tag="dg0", name="dg0"),
                    rpool.tile([2, 512], F32, tag="dg1", name="dg1")]
            rgrp = [rpool.tile([4, 512], F32, tag="rg0", name="rg0"),
                    rpool.tile([2, 512], F32, tag="rg1", name="rg1")]

            def emit_scores(h):
                p, sl = h // 2, 64 * (h % 2)
                sc = ps_sc.tile([128, 1536], F32, tag="sc", name="sc")
                for j in range(4):
                    # one MM per key-block: [keys 128j.., queries 128j..512)
                    nc.tensor.matmul(
                        sc[:, P_OFF[j]:P_OFF[j] + P_W[j]],
                        kt[p][sl:sl + HS, 128 * j:128 * (j + 1)],
                        qt[p][sl:sl + HS, 128 * j:512],
                        start=True, stop=True,
                    )
                eT = epool.tile([128, 1280], BF16, tag="eT", name="eT")
                nc.scalar.activation(eT[:, 0:EXP_END], sc[:, 0:EXP_END], AF.Exp)
                # causal mask on the in-chunk diagonal blocks: j0@0, j1@512
                # (stride 512), then j3@896 + j2@1024 (contiguous 256)
                m2 = eT[:, 0:640].rearrange("p (a b) -> p a b", b=128)[:, 0::4]
                t2 = tri4[:, 0:256].rearrange("p (a b) -> p a b", b=128)
                nc.gpsimd.tensor_mul(m2, m2, t2)
                nc.gpsimd.tensor_mul(eT[:, 896:1152], eT[:, 896:1152],
                                     tri4[:, 0:256])
                eT_b[h] = eT

            def emit_rect_o(h):
                eT = eT_b[h]
                ops = ps_po.tile([128, 512], F32, tag="po", name="ops")
                ops_b[h] = ops

            def emit_diag_o(h):
                eT, ops = eT_b[h], ops_b[h]
                for j in range(4):
                    nc.tensor.matmul(
                        ops[0:55, 128 * j:512],
                        vt[j][:, 64 * h: 64 * h + 55],
                        eT[:, P_OFF[j]:P_OFF[j] + P_W[j]],
                        start=(j == 0), stop=(j == 3),
                    )
                osb = rpool.tile([55, 512], BF16, tag=f"osb{h}", name=f"osb{h}")
                nc.vector.tensor_copy(osb[:], ops[0:55, :])
                osb_b[h] = osb
                if h < H - 1:
                    # gather the denominator row (bf16->f32 casting SWDGE)
                    # into the group tile; one reciprocal per group
                    g, gi = (0, h) if h < 4 else (1, h - 4)
                    nc.gpsimd.dma_start(dgrp[g][gi:gi + 1, :], osb[0:1, :])
                    if gi == (3 if g == 0 else 1):
                        nc.vector.reciprocal_approx_fast(rgrp[g][:], dgrp[g][:])
                else:
                    r6 = rpool.tile([1, 512], F32, tag="r6", name="r6")
                    nc.vector.reciprocal_approx_fast(r6[:], ops[0:1, :])
                    # last head gates Wo: broadcast via a PE outer-product
                    rbp = ps_sc.tile([128, 1536], F32, tag="sc", name="rb_ps")
                    nc.tensor.matmul(rbp[0:55, 0:512],
                                     ones32[0:1, 0:55].bitcast(mybir.dt.float32r),
                                     r6[:].bitcast(mybir.dt.float32r),
                                     start=True, stop=True)
                    ot = otpool.tile([55, 512], BF16, tag=f"ot{h}", name=f"ot{h}")
                    nc.vector.tensor_mul(ot[:], osb[:], rbp[0:55, 0:512])
                    ot_b[h] = ot

            def emit_normalize(h):
                g, gi = (0, h) if h < 4 else (1, h - 4)
                rb = rpool.tile([55, 512], BF16, tag="rb", name="rb")
                nc.gpsimd.dma_start(
                    rb[:], rgrp[g][gi:gi + 1, :].unsqueeze(1)
                    .to_broadcast([1, 55, 512]))
                ot = otpool.tile([55, 512], BF16, tag=f"ot{h}", name=f"ot{h}")
                nc.vector.tensor_mul(ot[:], osb_b[h][:], rb[:])
                ot_b[h] = ot

            for h in range(H + 2):
                if h < H:
                    emit_scores(h)
                if 1 <= h <= H:
                    emit_rect_o(h - 1)
                if 2 <= h <= H + 1:
                    emit_diag_o(h - 2)
                if h == H:
                    for hh in range(4):
                        emit_normalize(hh)
                if h == H + 1:
                    for hh in range(4, 6):
                        emit_normalize(hh)

            # next batch's LN1 apply: ACT is idle during Wo
            if b + 1 < n_b:
                xn_b = apply_ln(xin_b, rs_b, nmr_b)

            # ---- attention out-proj + residual -> x2 (SBUF) ----
            for j in range(4):
                ps = ps_po.tile([128, 512], F32, tag="po", name="ps_wo")
                for h in range(H):
                    nc.tensor.matmul(
                        ps[:, 0:D],
                        ot_b[h][:, 128 * j:128 * (j + 1)],
                        wo_sb[h][:],
                        start=(h == 0), stop=(h == H - 1 and not has_bias_o),
                    )
                if has_bias_o:
                    nc.tensor.matmul(ps[:, 0:D], ones_sb[:], bo_sb[:],
                                     start=False, stop=True)
                nc.vector.tensor_add(x2_sb[b][j][:], ps[:, 0:D], xin[j][:])
            # LN2 scale/shift for this batch (pure DVE)
            ln_stats([x2_sb[b][j] for j in range(4)], ln2_rs[b], ln2_nm[b])

          # ------------------- phase B: FFN -------------------
          xn2_b = apply_ln([x2_sb[0][j] for j in range(4)],
                           ln2_rs[0], ln2_nm[0])
          for b in range(n_b):
            xn2 = xn2_b
            xT2 = transpose_to_feature_major(xn2, evac="act")

            # FFN2 accumulators: 4 token-chunks in the 2 sc psum tiles
            # (bitcast to f32: [128, 1024] = 2 banks; chunks at 0 and 512)
            acc_t = [ps_sc.tile([128, 1536], F32, tag="sc", name="acc")
                     for _ in range(2)]
            acc = [acc_t[j // 2][:, 512 * (j % 2): 512 * (j % 2) + D]
                   for j in range(4)]
            for f in range(N_F):
                ps = ps_po.tile([128, 512], F32, tag="po", name="ps_f1")
                for d in range(N_D):
                    nc.tensor.matmul(
                        ps[:],
                        w1_sb[d][:, 128 * f:128 * (f + 1)],
                        xT2[d][:],
                        start=(d == 0), stop=(d == N_D - 1),
                    )
                hg = hpool.tile([128, 512], BF16, tag="hg", name="hg")
                nc.scalar.activation(hg[:], ps[:], AF.Gelu, bias=b1_sb[:, f:f + 1])
                if f == 5 and b + 1 < n_b:
                    xn2_b = apply_ln([x2_sb[b + 1][j] for j in range(4)],
                                     ln2_rs[b + 1], ln2_nm[b + 1])
                for j in range(4):
                    nc.tensor.matmul(
                        acc[j],
                        hg[:, 128 * j:128 * (j + 1)],
                        w2_sb[f][:],
                        start=(f == 0),
                        stop=(f == N_F - 1 and not has_bias_2),
                    )
            for j in range(4):
                t0 = 128 * (4 * b + j)
                if has_bias_2:
                    nc.tensor.matmul(acc[j], ones_sb[:], b2_sb[:],
                                     start=False, stop=True)
                ot = opool.tile([128, D], F32, tag="out", name="outt")
                nc.vector.tensor_add(ot[:], acc[j], x2_sb[b][j][:])
                nc.sync.dma_start(xdst_d[t0:t0 + 128, :], ot[:])

    nc.finalize()
    return nc


def preprocess(wq, bq, wk, bk, wv, bv, wo, bo, w1, b1, w2, b2,
               ln1_g, ln1_b, ln2_g, ln2_b):
    """Host-side folding: LN affine into weight matrices, attention scale into
    Q, V-bias into output bias; build padded/packed bf16 layouts."""
    import ml_dtypes
    f32 = np.float32
    bf16 = ml_dtypes.bfloat16
    args = [np.asarray(a, f32) for a in (wq, bq, wk, bk, wv, bv, wo, bo,
                                         w1, b1, w2, b2, ln1_g, ln1_b, ln2_g, ln2_b)]
    (wq, bq, wk, bk, wv, bv, wo, bo, w1, b1, w2, b2,
     ln1_g, ln1_b, ln2_g, ln2_b) = args
    scale = f32(HS) ** f32(-0.5)

    wq_pad = np.zeros((D, 512), f32)
    wk_pad = np.zeros((D, 512), f32)
    wv_pad = np.zeros((D, 512), f32)
    for h in range(H):
        wq_pad[:, 64 * h:64 * h + HS] = ln1_g[:, None] * wq[h] * scale
        wk_pad[:, 64 * h:64 * h + HS] = ln1_g[:, None] * wk[h]
        wv_pad[:, 64 * h + 1:64 * h + 1 + HS] = ln1_g[:, None] * wv[h]

    bq_eff = (bq + ln1_b @ wq).astype(f32)     # [H, HS]
    assert not np.any(bq_eff), "nonzero effective q bias not supported"
    # bk_eff shifts scores by a per-s constant -> cancelled by softmax; drop.

    bv_eff = (bv + ln1_b @ wv).astype(f32)     # [H, HS] -> folds into bo
    bo_eff = (bo + bv_eff.reshape(-1) @ wo).astype(f32)

    wo_pad = np.zeros((H, 55, D), f32)
    for h in range(H):
        wo_pad[h, 1:55, :] = wo[54 * h:54 * h + HS, :]

    w1_eff = (ln2_g[:, None] * w1).astype(f32)
    b1_eff = (b1 + ln2_b @ w1).astype(f32)
    b1c = np.ascontiguousarray(b1_eff.reshape(N_F, 128).T)   # [128, 18]

    def bf(a):
        return np.ascontiguousarray(a).astype(bf16)

    return dict(
        wq_pad=bf(wq_pad), wk_pad=bf(wk_pad), wv_pad=bf(wv_pad),
        wo_pad=bf(wo_pad),
        w1=bf(w1_eff), b1c=b1c, w2=bf(w2),
        bo=bf(bo_eff.reshape(1, D)), b2=bf(b2.reshape(1, D)),
        has_bias_o=bool(np.any(bo_eff)), has_bias_2=bool(np.any(b2)),
    )


def kernel(**inputs):
    x = np.asarray(inputs["x"], np.float32)
    w = preprocess(
        inputs["wq"], inputs["bq"], inputs["wk"], inputs["bk"],
        inputs["wv"], inputs["bv"], inputs["wo"], inputs["bo"],
        inputs["w1"], inputs["b1"], inputs["w2"], inputs["b2"],
        inputs["ln1_g"], inputs["ln1_b"], inputs["ln2_g"], inputs["ln2_b"],
    )
    has_bo, has_b2 = w.pop("has_bias_o"), w.pop("has_bias_2")
    nc = build_program(n_b=B_LOC, has_bias_o=has_bo, has_bias_2=has_b2)

    core_ids = list(range(N_CORES))
    in_maps = []
    for c in core_ids:
        m = dict(w)
        m["x"] = np.ascontiguousarray(
            x[B_LOC * c:B_LOC * (c + 1)].reshape(B_LOC * S, D))
        in_maps.append(m)

    res = run_bass_kernel_spmd(nc, in_maps, core_ids)
    global LAST_RESULTS
    LAST_RESULTS = res
    out = np.concatenate(
        [res.results[i]["out"].reshape(B_LOC, S, D) for i in range(N_CORES)], axis=0
    )
    return out.astype(np.float32)


# revision 27
# speedup vs baseline: 1.2750x; 1.2750x over previous
"""Trainium2 Bass kernel for a pre-LN transformer block (causal MHA + GELU FFN).

Problem: x[64, 512, 384], 7 heads x 54, FFN 2304. Sharded data-parallel over
batch across 8 NeuronCores (8 batches/core); no collectives needed.

v1 (bf16 rewrite of the fp32r baseline):
  - all matmul operands bf16 (fp32 accumulate in PSUM); rel-err budget 2e-2
  - LN rsqrt via exp(-0.5*ln(var+eps)) so the ACT engine table set stays on
    {ln,exp} for the whole attention phase and {gelu} for the FFN phase
    (2 table loads per rep instead of ~3 per batch)
  - per-head scores live in a 2-bank bf16 PSUM tile packed causally:
    4 diagonal 128-blocks at cols 0..512, rectangles at 512..896 /
    1024..1280 / 1280..1408; one exp() call [0..1408]; one 4-block
    triangular mask multiply on the diagonal region
  - x2 (post-attention residual) persists in SBUF; x is loaded once
  - FFN2 accumulates into the (phase-A scores) PSUM tiles bitcast to f32
"""

import numpy as np
from contextlib import ExitStack

import concourse.bass as bass
import concourse.bacc as bacc
import concourse.mybir as mybir
import concourse.tile as tile
from concourse import masks
from concourse.bass_utils import run_bass_kernel_spmd

# ---- problem constants (hardcoded per harness contract) ----
B, S, D = 64, 512, 384
H, HS = 7, 54
FFN = 6 * D  # 2304
EPS = 1e-5
N_CORES = 8
B_LOC = B // N_CORES          # 8 batches per core
F32 = mybir.dt.float32
BF16 = mybir.dt.bfloat16
FP8 = mybir.dt.float8e4
W_SCALE = 64.0                # fp8 weight prescale (w ~ 0.02 is subnormal in e4m3)
AF = mybir.ActivationFunctionType
ALU = mybir.AluOpType

N_D = D // 128                # 3 d-chunks
N_F = FFN // 128              # 18 ffn-chunks
PAIRS = (H + 1) // 2          # 4 head-pair groups (last has 1 head)

# packed causal-score layout inside a [128, 1536] f32 PSUM tile (3 banks;
# f32 bank = 512 elements -> matmul outputs must not cross 512-col lines).
# chunk j holds scores for key-block j vs queries s in [128j, 512) --
# width 512-128j, diagonal 128-block at the chunk start:
#   j0 [0..512) bank0 | j1 [512..896) + j3 [896..1024) bank1 | j2 [1024..1280) bank2
P_OFF = (0, 512, 1024, 896)
P_W = (512, 384, 256, 128)
EXP_END = 1280                # exp() covers [0, 1280) in one call

LAST_RESULTS = None


def build_program(n_b=B_LOC, has_bias_o=False, has_bias_2=False, n_reps=1):
    nc = bacc.Bacc()
    NTOK = n_b * S

    x_d = nc.declare_dram_parameter("x", [NTOK, D], F32, isOutput=False)
    wq_d = nc.declare_dram_parameter("wq_pad", [D, 512], BF16, isOutput=False)
    wk_d = nc.declare_dram_parameter("wk_pad", [D, 512], BF16, isOutput=False)
    wv_d = nc.declare_dram_parameter("wv_pad", [D, 512], BF16, isOutput=False)
    wo_d = nc.declare_dram_parameter("wo_pad", [H, 55, D], BF16, isOutput=False)
    w1_d = nc.declare_dram_parameter("w1dr", [128, 4 * FFN], FP8, isOutput=False)
    w2_d = nc.declare_dram_parameter("w2dr", [N_F // 2, 128, 2 * D], FP8, isOutput=False)
    b1_d = nc.declare_dram_parameter("b1c", [128, N_F], F32, isOutput=False)
    bo_d = nc.declare_dram_parameter("bo", [1, D], BF16, isOutput=False)
    b2_d = nc.declare_dram_parameter("b2", [1, D], BF16, isOutput=False)
    out_d = nc.declare_dram_parameter("out", [NTOK, D], F32, isOutput=True)

    with tile.TileContext(nc) as tc, ExitStack() as ctx, \
            nc.allow_low_precision(reason="bf16 kernel; rel-err gate 2e-2"):
        # ---------------- persistent pools ----------------
        wpool = ctx.enter_context(tc.tile_pool(name="weights", bufs=1))
        wq_sb = [wpool.tile([128, 512], BF16, tag=f"wq{d}", name=f"wq{d}") for d in range(N_D)]
        wk_sb = [wpool.tile([128, 512], BF16, tag=f"wk{d}", name=f"wk{d}") for d in range(N_D)]
        wv_sb = [wpool.tile([128, 512], BF16, tag=f"wv{d}", name=f"wv{d}") for d in range(N_D)]
        wo_sb = [wpool.tile([55, D], BF16, tag=f"wo{h}", name=f"wo{h}") for h in range(H)]
        w1_sb = wpool.tile([128, 4, FFN], FP8, tag="w1dr", name="w1dr")
        w2_sb = [wpool.tile([128, 2, D], FP8, tag=f"w2{f}", name=f"w2{f}")
                 for f in range(N_F // 2)]
        b1_sb = wpool.tile([128, N_F], F32, tag="b1")
        bo_sb = wpool.tile([1, D], BF16, tag="bo")
        b2_sb = wpool.tile([1, D], BF16, tag="b2")
        eps_sb = wpool.tile([128, 1], F32, tag="eps")
        ones32 = wpool.tile([1, 128], F32, tag="ones32")
        magic_sb = wpool.tile([128, 4], mybir.dt.uint32, tag="magic")
        ones_sb = wpool.tile([1, 128], BF16, tag="ones")
        tri4 = wpool.tile([128, 512], BF16, tag="tri4")
        identity = wpool.tile([128, 128], BF16, tag="ident")

        # x2 persists in SBUF between the attention and FFN phases
        x2pool = ctx.enter_context(tc.tile_pool(name="x2", bufs=1))
        x2_sb = [[x2pool.tile([128, D], F32, tag=f"x2_{b}_{j}", name=f"x2_{b}_{j}")
                  for j in range(4)] for b in range(n_b)]
        ln2_rs = [x2pool.tile([128, 4], F32, tag=f"rs2_{b}", name=f"rs2_{b}") for b in range(n_b)]
        ln2_nm = [x2pool.tile([128, 4], F32, tag=f"nm2_{b}", name=f"nm2_{b}") for b in range(n_b)]

        # ---------------- streaming pools ----------------
        xpool = ctx.enter_context(tc.tile_pool(name="xin", bufs=12))
        stpool = ctx.enter_context(tc.tile_pool(name="stats", bufs=4))
        xnpool = ctx.enter_context(tc.tile_pool(name="xn", bufs=8))
        xTpool = ctx.enter_context(tc.tile_pool(name="xT", bufs=2))
        qkpool = ctx.enter_context(tc.tile_pool(name="qk", bufs=2))
        vpool = ctx.enter_context(tc.tile_pool(name="v", bufs=8))
        epool = ctx.enter_context(tc.tile_pool(name="expT", bufs=3))
        rpool = ctx.enter_context(tc.tile_pool(name="recip", bufs=3))
        otpool = ctx.enter_context(tc.tile_pool(name="oT", bufs=2))
        hpool = ctx.enter_context(tc.tile_pool(name="hgelu", bufs=4))
        opool = ctx.enter_context(tc.tile_pool(name="outt", bufs=4))

        # prologue DMAs: batch-0 x first, then phase-A weights
        xpre = {}
        for j in range(4):
            xt = xpool.tile([128, D], F32, tag="x", name="xt")
            nc.sync.dma_start(xt[:], x_d[128 * j:128 * (j + 1), :])
            xpre[(0, j)] = xt
        for d in range(N_D):
            nc.sync.dma_start(wq_sb[d][:], wq_d[128 * d:128 * (d + 1), :])
            nc.sync.dma_start(wk_sb[d][:], wk_d[128 * d:128 * (d + 1), :])
            nc.sync.dma_start(wv_sb[d][:], wv_d[128 * d:128 * (d + 1), :])
        for h in range(H):
            nc.sync.dma_start(wo_sb[h][:], wo_d[h])
        nc.sync.dma_start(b1_sb[:], b1_d[:])
        nc.sync.dma_start(bo_sb[:], bo_d[:])
        nc.sync.dma_start(b2_sb[:], b2_d[:])
        nc.any.memset(eps_sb[:], EPS)
        nc.any.memset(magic_sb[:].bitcast(F32), np.uint32(0x5F3759DF).view(np.float32))
        nc.any.memset(ones_sb[:], 1.0)
        nc.any.memset(ones32[:], 1.0)
        masks.make_identity(nc, identity[:])
        for j in range(4):
            masks.make_upper_triangular(nc, tri4[:, 128 * j:128 * (j + 1)],
                                        val=1.0, diag=True)


        # prefetch batch-1 x ahead of the FFN weights
        if n_b > 1:
            for j in range(4):
                t0 = 128 * (4 + j)
                xt = xpool.tile([128, D], F32, tag="x", name="xt")
                nc.sync.dma_start(xt[:], x_d[t0:t0 + 128, :])
                xpre[(1, j)] = xt

        # second wave: FFN weights (queued behind phase-A essentials)
        nc.sync.dma_start(w1_sb[:].rearrange("p k m -> p (k m)"), w1_d[:])
        for f in range(N_F // 2):
            nc.sync.dma_start(w2_sb[f][:].rearrange("p k m -> p (k m)"), w2_d[f])

        dpool = ctx.enter_context(tc.tile_pool(name="dram", bufs=1, space="DRAM"))
        chain = [dpool.tile([NTOK, D], F32, tag=f"chain{i}", name=f"chain{i}")
                 for i in range(max(n_reps - 1, 0))]


        # PSUM budget (8 banks): sc 2x3 + shared proj/o 2x1
        ps_sc = ctx.enter_context(tc.tile_pool(name="ps_sc", bufs=2, space="PSUM"))
        ps_po = ctx.enter_context(tc.tile_pool(name="ps_po", bufs=2, space="PSUM"))

        def ln_stats(src_tiles, rs, nmr):
            """LN scale/shift for 4 token tiles -> rs/nmr [128, 4].
            rsqrt(var+eps) entirely on DVE (bit-trick seed + 2 Newton
            steps) so the ACT table set never changes mid-phase."""
            mv = stpool.tile([128, 8], F32, tag="mv", name="mv")
            for j in range(4):
                st6 = stpool.tile([128, 6], F32, tag="st6", name="st6")
                nc.vector.bn_stats(st6[:], src_tiles[j][:])
                nc.vector.bn_aggr(mv[:, 2 * j:2 * j + 2], st6[:])
            mv3 = mv[:].rearrange("p (j two) -> p j two", two=2)
            # rsqrt(var+eps) on DVE only: fast-inverse-sqrt seed + 2 Newton
            # steps (keeps the ACT table on {exp}/{gelu} all rep long)
            u = stpool.tile([128, 4], F32, tag="u", name="u")
            vh = stpool.tile([128, 4], F32, tag="vh", name="vh")
            yy = stpool.tile([128, 4], F32, tag="yy", name="yy")
            nc.vector.tensor_scalar(u[:], mv3[:, :, 1], EPS, None, op0=ALU.add)
            nc.vector.tensor_scalar(vh[:], u[:], 0.5, None, op0=ALU.mult)
            ui = u[:].bitcast(mybir.dt.uint32)
            nc.vector.tensor_scalar(ui, ui, 1, None, op0=ALU.logical_shift_right)
            nc.vector.tensor_tensor(rs[:].bitcast(mybir.dt.uint32), magic_sb[:],
                                    ui, op=ALU.subtract)
            for _ in range(2):
                nc.vector.tensor_mul(yy[:], rs[:], rs[:])
                nc.vector.tensor_mul(yy[:], yy[:], vh[:])
                nc.vector.tensor_scalar(yy[:], yy[:], -1.0, 1.5,
                                        op0=ALU.mult, op1=ALU.add)
                nc.vector.tensor_mul(rs[:], rs[:], yy[:])
            for j in range(4):
                # nmr = -(mu * rsig)
                nc.vector.tensor_scalar(nmr[:, j:j + 1], mv3[:, j, 0].unsqueeze(-1),
                                        rs[:, j:j + 1], -1.0,
                                        op0=ALU.mult, op1=ALU.mult)

        def apply_ln(src_tiles, rs, nmr, dtype=BF16):
            """xn = x*rs + nmr on ACT (Identity is in every table set)."""
            xn_tiles = []
            for j in range(4):
                xn = xnpool.tile([128, D], dtype, tag="xn", name="xn")
                nc.scalar.activation(xn[:], src_tiles[j][:], AF.Identity,
                                     bias=nmr[:, j:j + 1], scale=rs[:, j:j + 1])
                xn_tiles.append(xn)
            return xn_tiles

        def transpose_to_feature_major(xn_tiles, evac="dve"):
            """4x [128, D] token-major bf16 -> 3x [128, 512] feature-major."""
            xT = []
            for d in range(N_D):
                ps = ps_po.tile([128, 512], F32, tag="po", name="ps_t")
                psb = ps[:].bitcast(BF16)   # transpose out must match in dtype
                for j in range(4):
                    nc.tensor.transpose(
                        psb[:, 128 * j:128 * (j + 1)],
                        xn_tiles[j][:, 128 * d:128 * (d + 1)],
                        identity[:],
                    )
                t = xTpool.tile([128, 512], BF16, tag=f"xT{d}", name=f"xT{d}")
                if evac == "dve":
                    nc.vector.tensor_copy(t[:], psb[:, 0:512])
                else:
                    nc.scalar.copy(t[:], psb[:, 0:512])
                xT.append(t)
            return xT

        # ======================= per-rep =======================
        for rep in range(n_reps):
          xsrc_d = x_d if rep == 0 else chain[rep - 1]
          xdst_d = out_d if rep == n_reps - 1 else chain[rep]

          # ------------------- phase A: attention -------------------
          def load_x(b):
              xin = []
              for j in range(4):
                  t0 = 128 * (4 * b + j)
                  if rep == 0 and (b, j) in xpre:
                      xin.append(xpre[(b, j)])
                      continue
                  xt = xpool.tile([128, D], F32, tag="x", name="xt")
                  nc.sync.dma_start(xt[:], xsrc_d[t0:t0 + 128, :])
                  xin.append(xt)
              return xin

          def ln1_stats(xin):
              rs = stpool.tile([128, 4], F32, tag="rs", name="rs")
              nmr = stpool.tile([128, 4], F32, tag="nmr", name="nmr")
              ln_stats(xin, rs, nmr)
              return rs, nmr

          xin_b = load_x(0)
          rs_b, nmr_b = ln1_stats(xin_b)
          xn_b = apply_ln(xin_b, rs_b, nmr_b)
          for b in range(n_b):
            xin, xn_tiles = xin_b, xn_b
            xT = transpose_to_feature_major(xn_tiles)

            # Q^T / K^T per head-pair: [54, 512] at partitions 0-53/64-117
            qt, kt = [], []
            for p in range(PAIRS):
                m = 118 if p < PAIRS - 1 else 54
                for (dst_list, w_sb, tg) in ((qt, wq_sb, "q"), (kt, wk_sb, "k")):
                    ps = ps_po.tile([128, 512], F32, tag="po", name="ps_qk")
                    for d in range(N_D):
                        nc.tensor.matmul(
                            ps[0:m, :],
                            w_sb[d][:, 128 * p:128 * p + m],
                            xT[d][:],
                            start=(d == 0), stop=(d == N_D - 1),
                        )
                    t = qkpool.tile([128, 512], BF16, tag=f"{tg}{p}", name=f"{tg}{p}")
                    if tg == "q":
                        nc.scalar.copy(t[0:m, :], ps[0:m, :])
                    else:
                        nc.vector.tensor_copy(t[0:m, :], ps[0:m, :])
                    dst_list.append(t)

            # V token-major with a ones column per head
            vt = []
            for j in range(4):
                ps = ps_po.tile([128, 512], F32, tag="po", name="ps_v")
                for d in range(N_D):
                    nc.tensor.matmul(
                        ps[:],
                        xT[d][:, 128 * j:128 * (j + 1)],
                        wv_sb[d][:],
                        start=(d == 0), stop=(d == N_D - 1),
                    )
                t = vpool.tile([128, 512], BF16, tag="v", name="vt")
                nc.gpsimd.memset(t[:], 1.0)
                src = ps[:, 0:448].rearrange("p (h c) -> p h c", h=H)[:, :, 1:55]
                dst = t[:, 0:448].rearrange("p (h c) -> p h c", h=H)[:, :, 1:55]
                nc.scalar.copy(dst, src)
                vt.append(t)

            # next batch's LN1 stats: DVE is idle during this batch's heads
            if b + 1 < n_b:
                xin_b = load_x(b + 1)
                rs_b, nmr_b = ln1_stats(xin_b)

            # ---- per-head attention, software-pipelined on PE:
            #   iter h emits: scores(h) | rect-o(h-1) | diag-o(h-2)
            # so the Pool-mask latency never blocks the PE stream.
            ot_b = [None] * H
            eT_b = [None] * H
            ops_b = [None] * H

            def emit_scores(h):
                p, sl = h // 2, 64 * (h % 2)
                sc = ps_sc.tile([128, 1536], F32, tag="sc", name="sc")
                for j in range(4):
                    # one MM per key-block: [keys 128j.., queries 128j..512)
                    nc.tensor.matmul(
                        sc[:, P_OFF[j]:P_OFF[j] + P_W[j]],
                        kt[p][sl:sl + HS, 128 * j:128 * (j + 1)],
                        qt[p][sl:sl + HS, 128 * j:512],
                        start=True, stop=True,
                    )
                eT = epool.tile([128, 1280], BF16, tag="eT", name="eT")
                nc.scalar.activation(eT[:, 0:EXP_END], sc[:, 0:EXP_END], AF.Exp)
                # causal mask on the in-chunk diagonal blocks: j0@0, j1@512
                # (stride 512), then j3@896 + j2@1024 (contiguous 256)
                m2 = eT[:, 0:640].rearrange("p (a b) -> p a b", b=128)[:, 0::4]
                t2 = tri4[:, 0:256].rearrange("p (a b) -> p a b", b=128)
                nc.gpsimd.tensor_mul(m2, m2, t2)
                nc.gpsimd.tensor_mul(eT[:, 896:1152], eT[:, 896:1152],
                                     tri4[:, 0:256])
                eT_b[h] = eT

            def emit_rect_o(h):
                eT = eT_b[h]
                ops = ps_po.tile([128, 512], F32, tag="po", name="ops")
                ops_b[h] = ops

            def emit_diag_o(h):
                eT, ops = eT_b[h], ops_b[h]
                for j in range(4):
                    nc.tensor.matmul(
                        ops[0:55, 128 * j:512],
                        vt[j][:, 64 * h: 64 * h + 55],
                        eT[:, P_OFF[j]:P_OFF[j] + P_W[j]],
                        start=(j == 0), stop=(j == 3),
                    )
                osb = rpool.tile([55, 512], BF16, tag="osb", name="osb")
                nc.vector.tensor_copy(osb[:], ops[0:55, :])
                r = rpool.tile([1, 512], F32, tag="r", name="r")
                nc.vector.reciprocal_approx_fast(r[:], ops[0:1, :])
                ot = otpool.tile([55, 512], BF16, tag=f"ot{h}", name=f"ot{h}")
                if h == H - 1:
                    # last head gates Wo: broadcast the reciprocal row via a
                    # PE outer-product (fast) instead of the DMA broadcast
                    rb16 = rpool.tile([1, 512], BF16, tag="rb16", name="rb16")
                    nc.vector.tensor_copy(rb16[:], r[:])
                    rbp = ps_sc.tile([128, 1536], F32, tag="sc", name="rb_ps")
                    nc.tensor.matmul(rbp[0:55, 0:512], ones_sb[0:1, 0:55],
                                     rb16[:], start=True, stop=True)
                    nc.vector.tensor_mul(ot[:], osb[:], rbp[0:55, 0:512])
                else:
                    # bf16 broadcast via the casting software DGE (Pool)
                    rb = rpool.tile([55, 512], BF16, tag="rb", name="rb")
                    nc.gpsimd.dma_start(
                        rb[:], r[:].unsqueeze(1).to_broadcast([1, 55, 512]))
                    nc.vector.tensor_mul(ot[:], osb[:], rb[:])
                ot_b[h] = ot

            for h in range(H + 2):
                if h < H:
                    emit_scores(h)
                if 1 <= h <= H:
                    emit_rect_o(h - 1)
                if 2 <= h <= H + 1:
                    emit_diag_o(h - 2)


            # next batch's LN1 apply: ACT is idle during Wo
            if b + 1 < n_b:
                xn_b = apply_ln(xin_b, rs_b, nmr_b)

            # ---- attention out-proj + residual -> x2 (SBUF) ----
            for j in range(4):
                ps = ps_po.tile([128, 512], F32, tag="po", name="ps_wo")
                for h in range(H):
                    nc.tensor.matmul(
                        ps[:, 0:D],
                        ot_b[h][:, 128 * j:128 * (j + 1)],
                        wo_sb[h][:],
                        start=(h == 0), stop=(h == H - 1 and not has_bias_o),
                    )
                if has_bias_o:
                    nc.tensor.matmul(ps[:, 0:D], ones_sb[:], bo_sb[:],
                                     start=False, stop=True)
                nc.vector.tensor_add(x2_sb[b][j][:], ps[:, 0:D], xin[j][:])
            # LN2 scale/shift for this batch (pure DVE)
            ln_stats([x2_sb[b][j] for j in range(4)], ln2_rs[b], ln2_nm[b])

          # ------------------- phase B: FFN (fp8 DoubleRow) -------------------
          def xpose_fp8(xn_tiles):
              """xn2 fp8 -> DoubleRow layout [128, 4, 512] (k=3 zeroed)."""
              t = xTpool.tile([128, 4, 512], FP8, tag="xT2", name="xT2")
              nc.gpsimd.memset(t[:, 3, :], 0.0)
              for d in range(N_D):
                  ps = ps_po.tile([128, 512], F32, tag="po", name="ps_t")
                  psb = ps[:].bitcast(FP8)
                  for j in range(4):
                      nc.tensor.transpose(
                          psb[:, 128 * j:128 * (j + 1)],
                          xn_tiles[j][:, 128 * d:128 * (d + 1)],
                          identity[:],
                      )
                  nc.scalar.copy(t[:, d, :], psb[:, 0:512])
              return t

          xn2_b = apply_ln([x2_sb[0][j] for j in range(4)],
                           ln2_rs[0], ln2_nm[0], dtype=FP8)
          for b in range(n_b):
            xT2 = xpose_fp8(xn2_b)

            # FFN2 accumulators: 4 token-chunks in the 2 sc psum tiles
            acc_t = [ps_sc.tile([128, 1536], F32, tag="sc", name="acc")
                     for _ in range(2)]
            acc = [acc_t[j // 2][:, 512 * (j % 2): 512 * (j % 2) + D]
                   for j in range(4)]
            for f in range(N_F):
                if f % 2 == 0:
                    hg = hpool.tile([128, 2, 512], FP8, tag="hg", name="hg")
                ps = ps_po.tile([128, 512], F32, tag="po", name="ps_f1")
                for k in range(2):
                    nc.tensor.matmul(
                        ps[:],
                        w1_sb[:, 2 * k:2 * k + 2, 128 * f:128 * (f + 1)],
                        xT2[:, 2 * k:2 * k + 2, :],
                        start=(k == 0), stop=(k == 1),
                        perf_mode=mybir.MatmulPerfMode.DoubleRow,
                    )
                # FFN1 psum is W_SCALE*h1; gelu's input scale undoes it
                nc.scalar.activation(hg[:, f % 2, :], ps[:], AF.Gelu,
                                     bias=b1_sb[:, f:f + 1], scale=1.0 / W_SCALE)
                if f == 5 and b + 1 < n_b:
                    xn2_b = apply_ln([x2_sb[b + 1][j] for j in range(4)],
                                     ln2_rs[b + 1], ln2_nm[b + 1], dtype=FP8)
                if f % 2 == 1:
                    for j in range(4):
                        nc.tensor.matmul(
                            acc[j],
                            hg[:, 0:2, 128 * j:128 * (j + 1)],
                            w2_sb[f // 2][:],
                            start=(f == 1), stop=(f == N_F - 1),
                            perf_mode=mybir.MatmulPerfMode.DoubleRow,
                        )
            for j in range(4):
                t0 = 128 * (4 * b + j)
                ot = opool.tile([128, D], F32, tag="out", name="outt")
                # acc holds W_SCALE*ffn (+W_SCALE*b2 folded in preprocess)
                nc.vector.tensor_scalar(ot[:], acc[j], 1.0 / W_SCALE, None,
                                        op0=ALU.mult)
                nc.vector.tensor_add(ot[:], ot[:], x2_sb[b][j][:])
                nc.sync.dma_start(xdst_d[t0:t0 + 128, :], ot[:])

    nc.finalize()
    return nc


def preprocess(wq, bq, wk, bk, wv, bv, wo, bo, w1, b1, w2, b2,
               ln1_g, ln1_b, ln2_g, ln2_b):
    """Host-side folding: LN affine into weight matrices, attention scale into
    Q, V-bias into output bias; build padded/packed bf16 layouts."""
    import ml_dtypes
    f32 = np.float32
    bf16 = ml_dtypes.bfloat16
    args = [np.asarray(a, f32) for a in (wq, bq, wk, bk, wv, bv, wo, bo,
                                         w1, b1, w2, b2, ln1_g, ln1_b, ln2_g, ln2_b)]
    (wq, bq, wk, bk, wv, bv, wo, bo, w1, b1, w2, b2,
     ln1_g, ln1_b, ln2_g, ln2_b) = args
    scale = f32(HS) ** f32(-0.5)

    wq_pad = np.zeros((D, 512), f32)
    wk_pad = np.zeros((D, 512), f32)
    wv_pad = np.zeros((D, 512), f32)
    for h in range(H):
        wq_pad[:, 64 * h:64 * h + HS] = ln1_g[:, None] * wq[h] * scale
        wk_pad[:, 64 * h:64 * h + HS] = ln1_g[:, None] * wk[h]
        wv_pad[:, 64 * h + 1:64 * h + 1 + HS] = ln1_g[:, None] * wv[h]

    bq_eff = (bq + ln1_b @ wq).astype(f32)     # [H, HS]
    assert not np.any(bq_eff), "nonzero effective q bias not supported"
    # bk_eff shifts scores by a per-s constant -> cancelled by softmax; drop.

    bv_eff = (bv + ln1_b @ wv).astype(f32)     # [H, HS] -> folds into bo
    bo_eff = (bo + bv_eff.reshape(-1) @ wo).astype(f32)

    wo_pad = np.zeros((H, 55, D), f32)
    for h in range(H):
        wo_pad[h, 1:55, :] = wo[54 * h:54 * h + HS, :]

    w1_eff = (ln2_g[:, None] * w1).astype(f32)
    b1_eff = (b1 + ln2_b @ w1).astype(f32)
    b1c = np.ascontiguousarray(b1_eff.reshape(N_F, 128).T)   # [128, 18]

    def bf(a):
        return np.ascontiguousarray(a).astype(bf16)

    fp8 = ml_dtypes.float8_e4m3fn
    ws = f32(64.0)  # keep in sync with W_SCALE
    # w1 DoubleRow layout [128, 4, FFN]: w1dr[p, k, m] = ws*w1[128k+p, m]
    w1dr = np.zeros((128, 4, FFN), f32)
    for k in range(3):
        w1dr[:, k, :] = ws * w1_eff[128 * k:128 * (k + 1), :]
    # w2 DoubleRow pair tiles [9][128, 2, D]
    w2dr = np.zeros((N_F // 2, 128, 2, D), f32)
    for pr in range(N_F // 2):
        for k in range(2):
            w2dr[pr, :, k, :] = ws * w2[128 * (2 * pr + k):128 * (2 * pr + k + 1), :]
    assert not np.any(b2), "nonzero b2 not supported in fp8 FFN path"

    return dict(
        wq_pad=bf(wq_pad), wk_pad=bf(wk_pad), wv_pad=bf(wv_pad),
        wo_pad=bf(wo_pad),
        w1dr=np.ascontiguousarray(w1dr.reshape(128, 4 * FFN)).astype(fp8),
        b1c=b1c,
        w2dr=np.ascontiguousarray(w2dr.reshape(N_F // 2, 128, 2 * D)).astype(fp8),
        bo=bf(bo_eff.reshape(1, D)), b2=bf(b2.reshape(1, D)),
        has_bias_o=bool(np.any(bo_eff)), has_bias_2=False,
    )


def kernel(**inputs):
    x = np.asarray(inputs["x"], np.float32)
    w = preprocess(
        inputs["wq"], inputs["bq"], inputs["wk"], inputs["bk"],
        inputs["wv"], inputs["bv"], inputs["wo"], inputs["bo"],
        inputs["w1"], inputs["b1"], inputs["w2"], inputs["b2"],
        inputs["ln1_g"], inputs["ln1_b"], inputs["ln2_g"], inputs["ln2_b"],
    )
    has_bo, has_b2 = w.pop("has_bias_o"), w.pop("has_bias_2")
    nc = build_program(n_b=B_LOC, has_bias_o=has_bo, has_bias_2=has_b2)

    core_ids = list(range(N_CORES))
    in_maps = []
    for c in core_ids:
        m = dict(w)
        m["x"] = np.ascontiguousarray(
            x[B_LOC * c:B_LOC * (c + 1)].reshape(B_LOC * S, D))
        in_maps.append(m)

    res = run_bass_kernel_spmd(nc, in_maps, core_ids)
    global LAST_RESULTS
    LAST_RESULTS = res
    out = np.concatenate(
        [res.results[i]["out"].reshape(B_LOC, S, D) for i in range(N_CORES)], axis=0
    )
    return out.astype(np.float32)
